# revision 50
# baseline (speedup 1.0000x reference)
import sys

if "/opt/trn_rl_repo" not in sys.path:
    sys.path.insert(0, "/opt/trn_rl_repo")

import numpy as np
import ml_dtypes

BF16 = ml_dtypes.bfloat16
E4M3 = ml_dtypes.float8_e4m3
B, S, H = 2, 2048, 4096
NH, NKV, D = 32, 8, 128
T = B * S
KBLK = H // 128  # 32
SCALE = float(D) ** -0.5
NCORES = 8
WS = 64.0  # weight quantization scale (wq/wk/wv/wo pre-scaled by 64)
# q,k carry 64x -> raw scores carry 4096x -> exp scale = SCALE/4096
# v carries 64x; ones-col 4.0 makes OT = 16*attn; y psum = 16*64 = 1024x
EXP_SCALE = SCALE / (WS * WS)
Y_SCALE = 1.0 / (16.0 * WS)

_NC = None


def build_nc():
    from concourse import bacc, tile, mybir

    dt = mybir.dt
    Act = mybir.ActivationFunctionType
    Alu = mybir.AluOpType
    DR = mybir.MatmulPerfMode.DoubleRow

    nc = bacc.Bacc("TRN2", target_bir_lowering=False, debug=False, num_devices=NCORES)

    hsh_d = nc.dram_tensor("hsh", [128, KBLK, T], dt.float8e4, kind="ExternalInput")
    hsl_d = nc.dram_tensor("hsl", [128, KBLK, T], dt.float8e4, kind="ExternalInput")
    cos_d = nc.dram_tensor("cosT", [128, T], dt.bfloat16, kind="ExternalInput")
    sin_d = nc.dram_tensor("sinT", [128, T], dt.bfloat16, kind="ExternalInput")
    wqh_d = nc.dram_tensor("wqh", [128, KBLK, 512], dt.float8e4, kind="ExternalInput")
    wql_d = nc.dram_tensor("wql", [128, KBLK, 512], dt.float8e4, kind="ExternalInput")
    wkh_d = nc.dram_tensor("wkh", [128, KBLK, 128], dt.float8e4, kind="ExternalInput")
    wkl_d = nc.dram_tensor("wkl", [128, KBLK, 128], dt.float8e4, kind="ExternalInput")
    wvh_d = nc.dram_tensor("wvh", [128, KBLK, 128], dt.float8e4, kind="ExternalInput")
    wvl_d = nc.dram_tensor("wvl", [128, KBLK, 128], dt.float8e4, kind="ExternalInput")
    woh_d = nc.dram_tensor("woh", [128, 4, H], dt.float8e4, kind="ExternalInput")
    wol_d = nc.dram_tensor("wol", [128, 4, H], dt.float8e4, kind="ExternalInput")
    y_d = nc.dram_tensor("y", [T, H], dt.bfloat16, kind="ExternalOutput")

    with tile.TileContext(nc) as tc:
        with tc.tile_pool(name="persist", bufs=1) as pp:
            Qts = [
                [pp.tile([128, 512], dt.bfloat16, name=f"q{h}_{t}") for t in range(8)]
                for h in range(4)
            ]
            Kts = [pp.tile([128, 512], dt.bfloat16, name=f"k{t}") for t in range(8)]
            # V augmented with a 4.0 column at free idx 128 (denominator trick;
            # 4.0 turns the 64x v scale into a 16x OT scale)
            Vts = [pp.tile([128, 132], dt.bfloat16, name=f"v{c}") for c in range(32)]
            OTh = [pp.tile([128, 4, 128], dt.float8e4, name=f"oh{i}") for i in range(32)]
            OTl = [pp.tile([128, 4, 128], dt.float8e4, name=f"ol{i}") for i in range(32)]
            for c in range(32):
                nc.vector.memset(Vts[c][:, 128:132], 4.0)
            # static lower-triangular mask: tri[p, q] = 1.0 iff q >= p
            tri_sb = pp.tile([128, 128], dt.bfloat16, name="tri")
            nc.vector.memset(tri_sb[:], 1.0)
            nc.gpsimd.affine_select(
                tri_sb[:],
                tri_sb[:],
                pattern=[[1, 128]],
                compare_op=Alu.is_ge,
                fill=0.0,
                base=0,
                channel_multiplier=-1,
            )

            # ---------------- Phase A: QKV projections (fp8 3-pass) + RoPE ---
            with (
                tc.tile_pool(name="aw", bufs=1) as aw,
                tc.tile_pool(name="slabp", bufs=2) as slabp,
                tc.tile_pool(name="ascr", bufs=3) as ascr,
                tc.tile_pool(name="psa", bufs=1, space="PSUM") as psa,
            ):
                wq_hi = aw.tile([128, KBLK, 512], dt.float8e4)
                wq_lo = aw.tile([128, KBLK, 512], dt.float8e4)
                wk_hi = aw.tile([128, KBLK, 128], dt.float8e4)
                wk_lo = aw.tile([128, KBLK, 128], dt.float8e4)
                wv_hi = aw.tile([128, KBLK, 128], dt.float8e4)
                wv_lo = aw.tile([128, KBLK, 128], dt.float8e4)
                cos_sb = aw.tile([128, T], dt.bfloat16)
                sin_sb = aw.tile([128, T], dt.bfloat16)

                def rope(hd, tb, src_ps):
                    # dst[:64] = x[:64]*cos[:64] - x[64:]*sin[:64]
                    # dst[64:] = x[64:]*cos[64:] + x[:64]*sin[64:]
                    c0, c1 = tb * 512, (tb + 1) * 512
                    xb = ascr.tile([128, 512], dt.bfloat16)
                    nc.scalar.activation(xb[:], src_ps[:], Act.Copy)
                    tmpc = ascr.tile([128, 512], dt.bfloat16)
                    tmps = ascr.tile([128, 512], dt.bfloat16)
                    nc.vector.tensor_mul(tmpc[:], xb[:], cos_sb[:, c0:c1])
                    nc.vector.tensor_mul(
                        tmps[0:64, :], src_ps[64:128, :], sin_sb[0:64, c0:c1]
                    )
                    nc.vector.tensor_mul(
                        tmps[64:128, :], src_ps[0:64, :], sin_sb[64:128, c0:c1]
                    )
                    if hd is None:
                        d_lo = Kts[tb][0:64, :]
                        d_hi = Kts[tb][64:128, :]
                    else:
                        d_lo = Qts[hd][tb][0:64, :]
                        d_hi = Qts[hd][tb][64:128, :]
                    nc.vector.tensor_sub(d_lo, tmpc[0:64, :], tmps[0:64, :])
                    nc.vector.tensor_add(d_hi, tmpc[64:128, :], tmps[64:128, :])

                pending_vt = []

                def flush_vt():
                    # Vts transposes are emitted AFTER the next slab's loads so
                    # the SP DMA queue never parks input loads behind them.
                    for dst, src in pending_vt:
                        nc.sync.dma_start_transpose(dst, src)
                    pending_vt.clear()

                for tb in range(8):
                    c0 = tb * 512
                    kp = psa.tile([128, 512], dt.float32)
                    vtp = psa.tile([128, 512], dt.float32)
                    qps = [
                        psa.tile([128, 512], dt.float32, name=f"qp{_h}")
                        for _h in range(4)
                    ]
                    for half in range(2):
                        sl_h = slabp.tile([128, 16, 512], dt.float8e4)
                        sl_l = slabp.tile([128, 16, 512], dt.float8e4)
                        hsli = slice(half * 16, (half + 1) * 16)
                        if tb == 0:
                            w0, w1 = (0, 16) if half == 0 else (16, 32)
                            if half == 1:
                                nc.sync.dma_start(
                                    wv_hi[:, w0:w1, :], wvh_d[:, w0:w1, :]
                                )
                                nc.sync.dma_start(sl_h[:], hsh_d[:, w0:w1, c0 : c0 + 512])
                            else:
                                nc.sync.dma_start(
                                    sl_h[:, 0:4, :],
                                    hsh_d[:, w0 : w0 + 4, c0 : c0 + 512],
                                )
                                nc.sync.dma_start(
                                    wv_hi[:, w0:w1, :], wvh_d[:, w0:w1, :]
                                )
                                nc.sync.dma_start(
                                    sl_h[:, 4:16, :],
                                    hsh_d[:, w0 + 4 : w1, c0 : c0 + 512],
                                )
                            nc.sync.dma_start(wv_lo[:, w0:w1, :], wvl_d[:, w0:w1, :])
                            nc.sync.dma_start(sl_l[:], hsl_d[:, w0:w1, c0 : c0 + 512])
                            nc.sync.dma_start(wk_hi[:, w0:w1, :], wkh_d[:, w0:w1, :])
                            nc.sync.dma_start(wk_lo[:, w0:w1, :], wkl_d[:, w0:w1, :])
                            nc.sync.dma_start(wq_hi[:, w0:w1, :], wqh_d[:, w0:w1, :])
                            nc.sync.dma_start(wq_lo[:, w0:w1, :], wql_d[:, w0:w1, :])
                            if half == 1:
                                nc.sync.dma_start(cos_sb[:], cos_d[:])
                                nc.sync.dma_start(sin_sb[:], sin_d[:])
                        else:
                            nc.sync.dma_start(sl_h[:], hsh_d[:, hsli, c0 : c0 + 512])
                            nc.sync.dma_start(sl_l[:], hsl_d[:, hsli, c0 : c0 + 512])
                        if half == 1:
                            flush_vt()

                        def dr3(ps_t, w_h, w_l, cs=None):
                            for ps in range(3):
                                lhs_t = w_h if ps != 1 else w_l
                                rhs_t = sl_h if ps != 2 else sl_l
                                for p in range(8):
                                    kb = half * 16 + 2 * p
                                    lhs = (
                                        lhs_t[:, kb : kb + 2, :]
                                        if cs is None
                                        else lhs_t[:, kb : kb + 2, cs : cs + 128]
                                    )
                                    nc.tensor.matmul(
                                        ps_t[:],
                                        lhs,
                                        rhs_t[:, 2 * p : 2 * p + 2, :],
                                        start=(half == 0 and ps == 0 and p == 0),
                                        stop=(half == 1 and ps == 2 and p == 7),
                                        perf_mode=DR,
                                    )

                        dr3(vtp, wv_hi, wv_lo)
                        dr3(kp, wk_hi, wk_lo)
                        for hd in range(4):
                            dr3(qps[hd], wq_hi, wq_lo, cs=hd * 128)
                    rope(None, tb, kp)
                    vt_sb = ascr.tile([128, 512], dt.bfloat16, bufs=2)
                    nc.vector.tensor_copy(vt_sb[:], vtp[:])
                    for s4 in range(4):
                        pending_vt.append(
                            (
                                Vts[tb * 4 + s4][:, 0:128],
                                vt_sb[:, s4 * 128 : (s4 + 1) * 128],
                            )
                        )
                    for hd in range(4):
                        rope(hd, tb, qps[hd])
                flush_vt()

            # ---------------- Phase B (attention) + woven Phase C (o_proj) ---
            with (
                tc.tile_pool(name="cw", bufs=1) as cw,
                tc.tile_pool(name="bpt", bufs=18) as bpt,
                tc.tile_pool(name="brp", bufs=3) as brp,
                tc.tile_pool(name="brc", bufs=8) as brc,
                tc.tile_pool(name="ota", bufs=2) as ota,
                tc.tile_pool(name="otb", bufs=2) as otb,
                tc.tile_pool(name="cy", bufs=3) as cy,
                tc.tile_pool(name="pss", bufs=2, space="PSUM") as pss,
                tc.tile_pool(name="pog", bufs=2, space="PSUM") as pog,
                tc.tile_pool(name="psy", bufs=2, space="PSUM") as psy,
            ):
                wo_hi = cw.tile([128, 4, H], dt.float8e4)
                wo_lo = cw.tile([128, 4, H], dt.float8e4)
                nc.sync.dma_start(wo_hi[:], woh_d[:])
                nc.sync.dma_start(wo_lo[:], wol_d[:])

                cqueue = []
                cstate = {"ysb": None, "units": 0, "emitted": 0}

                def emit_c_block(force=False):
                    if not cqueue:
                        return
                    i, cb, tag = cqueue[0]
                    if not force:
                        if cstate["units"] < tag + 4:
                            return

                    cqueue.pop(0)
                    cstate["emitted"] += 1
                    if cb == 0:
                        cstate["ysb"] = cy.tile([128, H], dt.bfloat16, name="ysb")
                    ysb = cstate["ysb"]
                    yp = psy.tile([128, 512], dt.float32, name="yp")
                    for hp in range(2):
                        for ps in range(3):
                            lhs_t = OTh[i] if ps != 1 else OTl[i]
                            rhs_t = wo_hi if ps != 2 else wo_lo
                            nc.tensor.matmul(
                                yp[:],
                                lhs_t[:, 2 * hp : 2 * hp + 2, :],
                                rhs_t[:, 2 * hp : 2 * hp + 2, cb * 512 : (cb + 1) * 512],
                                start=(hp == 0 and ps == 0),
                                stop=(hp == 1 and ps == 2),
                                perf_mode=DR,
                            )
                    nc.vector.tensor_copy(ysb[:, cb * 512 : (cb + 1) * 512], yp[:])
                    w = 1 if i >= 31 else (2 if i >= 30 else 8)
                    if cb % w == w - 1:
                        nc.sync.dma_start(
                            y_d[
                                i * 128 : (i + 1) * 128,
                                (cb - w + 1) * 512 : (cb + 1) * 512,
                            ],
                            ysb[:, (cb - w + 1) * 512 : (cb + 1) * 512],
                        )

                for g in range(4):
                    for b in range(2):
                        onat4 = [
                            ota.tile([128, 4, 128], dt.bfloat16, name=f"oa{_q}")
                            for _q in range(4)
                        ]
                        otb16 = [
                            otb.tile([128, 4, 128], dt.bfloat16, name=f"ob{_q}")
                            for _q in range(4)
                        ]
                        for hd in range(4):
                            q0 = b * S + g * 512
                            nj = 4 * g + 4
                            diag = list(range(4 * g, nj))
                            off = list(range(0, 4 * g))
                            first_j = off[0] if off else diag[0]
                            pts = {}
                            ptms = {}

                            def emit_st_exp(j, masked):
                                st = pss.tile([128, 512], dt.float32, name="st")
                                c0 = (j - 4 * g) * 128 if masked else 0
                                nc.tensor.matmul(
                                    st[:, c0:512],
                                    Kts[b * 4 + j // 4][
                                        :, (j % 4) * 128 : (j % 4 + 1) * 128
                                    ],
                                    Qts[hd][b * 4 + g][:, c0:512],
                                    start=True,
                                    stop=True,
                                )
                                pt = bpt.tile([128, 512], dt.bfloat16, name="pt")
                                nc.scalar.activation(
                                    pt[:, c0:512], st[:, c0:512], Act.Exp,
                                    scale=EXP_SCALE,
                                )
                                if masked:
                                    # triangular chunk: keep where qq >= p
                                    ptm = brp.tile([128, 128], dt.bfloat16, name="ptm")
                                    nc.vector.tensor_mul(
                                        ptm[:], pt[:, c0 : c0 + 128], tri_sb[:]
                                    )
                                    ptms[j] = ptm
                                pts[j] = pt

                            def emit_ot2(j, ogs, p):
                                dg = j - 4 * g
                                for qc in (2 * p, 2 * p + 1):
                                    if dg >= 0 and qc < dg:
                                        continue
                                    if dg >= 0 and qc == dg:
                                        src = ptms[j][:]
                                    else:
                                        src = pts[j][:, qc * 128 : (qc + 1) * 128]
                                    nc.tensor.matmul(
                                        ogs[qc - 2 * p][:, 0:129],
                                        src,
                                        Vts[b * 16 + j][:, 0:129],
                                        start=(j == first_j),
                                        stop=(dg == qc),
                                    )

                            def emit_norms(ogs, p):
                                for qc in (2 * p, 2 * p + 1):
                                    og = ogs[qc - 2 * p]
                                    rc = brc.tile([128, 1], dt.float32, name="rc")
                                    nc.vector.reciprocal(rc[:], og[:, 128:129])
                                    if g < 2:
                                        nc.vector.tensor_scalar_mul(
                                            onat4[qc][:, hd, :], og[:, 0:128], rc[:]
                                        )
                                    else:
                                        nc.scalar.activation(
                                            onat4[qc][:, hd, :],
                                            og[:, 0:128],
                                            Act.Copy,
                                            scale=rc[:],
                                        )

                            ogs_a = [
                                pog.tile([128, 132], dt.float32, name=f"og{_q}")
                                for _q in range(2)
                            ]
                            for j in diag:
                                emit_st_exp(j, True)
                            for idx, j in enumerate(off):
                                emit_st_exp(j, False)
                                if idx > 0:
                                    emit_c_block()
                                    emit_ot2(off[idx - 1], ogs_a, 0)
                            if off:
                                emit_c_block()
                                emit_ot2(off[-1], ogs_a, 0)
                            for j in diag:
                                emit_ot2(j, ogs_a, 0)
                            emit_norms(ogs_a, 0)
                            emit_c_block()
                            ogs_b = [
                                pog.tile([128, 132], dt.float32, name=f"og{_q}")
                                for _q in range(2)
                            ]
                            for j in off:
                                emit_ot2(j, ogs_b, 1)
                            for j in diag:
                                emit_ot2(j, ogs_b, 1)
                            emit_norms(ogs_b, 1)
                            emit_c_block()
                            emit_c_block()
                            cstate["units"] += 1
                        # transpose + quantize this group's 4 token-blocks
                        for qc in range(4):
                            nc.sync.dma_start_transpose(otb16[qc][:], onat4[qc][:])
                        for qc in range(4):
                            i = b * 16 + g * 4 + qc
                            nc.vector.tensor_copy(OTh[i][:], otb16[qc][:])
                            nc.vector.tensor_sub(
                                OTl[i][:], otb16[qc][:], OTh[i][:]
                            )
                        for ii in range(4):
                            for cb in range(8):
                                cqueue.append(
                                    (b * 16 + g * 4 + ii, cb, cstate["units"])
                                )
                while cqueue:
                    emit_c_block(force=True)

    nc.compile()
    return nc


def _hilo(x):
    hi = x.astype(E4M3)
    lo = (x - hi.astype(np.float32)).astype(E4M3)
    return hi, lo


def prep_inputs(inputs):
    hs = np.asarray(inputs["hidden_states"], np.float32)
    cos = np.asarray(inputs["cos"], np.float32)
    sin = np.asarray(inputs["sin"], np.float32)
    wq = np.asarray(inputs["wq"], np.float32) * np.float32(WS)
    wk = np.asarray(inputs["wk"], np.float32) * np.float32(WS)
    wv = np.asarray(inputs["wv"], np.float32) * np.float32(WS)
    wo = np.asarray(inputs["wo"], np.float32) * np.float32(WS)

    hsT = hs.reshape(T, H).T  # [H, T]
    hsT_p = hsT.reshape(KBLK, 128, T).transpose(1, 0, 2)
    hsh, hsl = _hilo(hsT_p)
    cosT = cos.transpose(2, 0, 1).reshape(128, T).astype(BF16)
    sinT = sin.transpose(2, 0, 1).reshape(128, T).astype(BF16)

    in_maps = []
    for c in range(NCORES):
        wq_c = wq[:, c * 512 : (c + 1) * 512]
        wk_c = wk[:, c * 128 : (c + 1) * 128]
        wv_c = wv[:, c * 128 : (c + 1) * 128]
        wo_c = wo[c * 512 : (c + 1) * 512, :]
        wqh, wql = _hilo(wq_c.reshape(KBLK, 128, 512).transpose(1, 0, 2))
        wkh, wkl = _hilo(wk_c.reshape(KBLK, 128, 128).transpose(1, 0, 2))
        wvh, wvl = _hilo(wv_c.reshape(KBLK, 128, 128).transpose(1, 0, 2))
        woh, wol = _hilo(wo_c.reshape(4, 128, H).transpose(1, 0, 2))
        in_maps.append(
            {
                "hsh": hsh,
                "hsl": hsl,
                "cosT": cosT,
                "sinT": sinT,
                "wqh": wqh,
                "wql": wql,
                "wkh": wkh,
                "wkl": wkl,
                "wvh": wvh,
                "wvl": wvl,
                "woh": woh,
                "wol": wol,
            }
        )
    return in_maps


def kernel(**inputs):
    global _NC
    from concourse.bass_utils import run_bass_kernel_spmd

    if _NC is None:
        _NC = build_nc()
    in_maps = prep_inputs(inputs)
    res = run_bass_kernel_spmd(_NC, in_maps, list(range(NCORES)))
    y = np.zeros((T, H), np.float64)
    for c in range(NCORES):
        y += res.results[c]["y"].astype(np.float64)
    y *= Y_SCALE  # undo the 16x OT * 64x wo scaling
    return y.reshape(B, S, H).astype(np.float32)
